# revision 1
# baseline (speedup 1.0000x reference)
"""Trainium2 Bass kernel for EntropySamplLoss.

Reference semantics (per image b):
  acts [N, P=320] viewed as [N, S=4, C=8, K=10] prototype groups
  ent[n, s, c] = normalized softmax entropy over the K protos of group (s, c)
  num[s, c]   = sum over pixels n with label c of ent[n, s, c]
  cnt[c]      = number of pixels with label c
  loss = mean over present (b, s, c) of num[s, c] / cnt[c]

Device kernel (data-parallel, one image per NeuronCore), v6:
  per chunk of 1024 pixels (tile [128 part, 2560], 8 px per partition):
    E    = exp(x)            -> bf16   (ACT)
    SY   = silu(x - m)       -> bf16   (ACT)  [silu(y) ~ y*e^y for y << 0,
                                        so sum_k x*e^x = e^m*sum SY + m*Z]
    Z    = tree-sum_k E                (DVE bf16 tensor_tensor at 2x mode)
    SS   = tree-sum_k SY               (DVE bf16 tensor_tensor at 2x mode)
    logZ = ln(Z)                       (ACT)
    rZ   = 1/Z                         (DVE reciprocal_approx_fast)
    meanx = (-SS) * rZ                 (DVE scalar_tensor_tensor)
    mask[px, (j,c)] = labels==c+1      (DVE is_equal vs PSUM-resident iota)
    stats1[(j,c),(j',sc|1)] += mask^T @ [logZ | 1]   (PE matmul, PSUM accum)
    stats2[(j,c),(j',sc)]   += mask^T @ meanx        (PE matmul, PSUM accum)
  host: diagonal j==j', ent = stats1 + e^m*stats2 - m*cnt, /ln(K),
  per-class means, final mean over present (image, scale, class) cells.

ACT ops are emitted in phase groups of PH chunks (all exps, then all silus,
then lns) so the activation-table switches between the exp/ln set and the
silu set amortize. GPSIMD is unused: the x*e^x multiply is replaced by the
silu identity, which also removes the GPSIMD<->DVE SBUF-port contention,
letting the bf16 2x-mode tensor_tensor trees replace the 1x tensor_reduce.
Measured: 388.5 us/core HW exec, rel err 1.1e-5 (first correct version was
727.7 us; DMA roofline ~233 us). Bottleneck: ACT ~94% busy -- the exp and
silu passes are 2.3 us/chunk each and irreducible in this formulation.
"""

import sys

if "/opt/trn_rl_repo" not in sys.path:
    sys.path.insert(0, "/opt/trn_rl_repo")

from contextlib import ExitStack

import numpy as np

import concourse.bacc as bacc
import concourse.bass as bass
import concourse.tile as tile
from concourse import mybir
from concourse.bass_utils import run_bass_kernel_spmd

# Problem shape (hardcoded per spec)
B, N, PP = 8, 65536, 320
S, C, K = 4, 8, 10
NCORES = 8

PX_PER_PART = 8          # pixels per partition ("j" slots)
PART = 128
PX_PER_CHUNK = PART * PX_PER_PART      # 1024
NCHUNK = N // PX_PER_CHUNK             # 64
FREE = PX_PER_PART * PP                # 2560
G = S * C                              # 32 groups per pixel
GF = PX_PER_PART * G                   # 256 group slots per partition
EW = G + 1                             # 33: ent cols + ones col
PH = 5                                 # chunks per ACT table-set phase group
MSHIFT = 12.0                          # silu(x-m) ~ (x-m)e^(x-m) shift

_CACHE = {}


def _patch_act_tables():
    """Make the combined exp+ln table set the only candidate for Exp/Ln so
    the table-load placement pass doesn't thrash between per-function sets."""
    import concourse.hw_specs as hw_specs

    tabs = hw_specs.get_activation_tables("gen3")
    E = mybir.ActivationFunctionType.Exp
    L = mybir.ActivationFunctionType.Ln
    for name, funcs in tabs.items():
        if name != "natural_log_exp_and_others":
            funcs.discard(E)
            funcs.discard(L)


def _tree_groupsum(nc, pool, src, out, tag):
    """out[p, g] (f32) = sum_k src[p, g, k] for K=10 bf16 groups, using
    tensor_tensor adds that hit the DVE 2x_1P bf16 mode (tensor_reduce is
    stuck at 1x)."""
    bf16 = mybir.dt.bfloat16
    s3 = src[:].rearrange("p (g k) -> p g k", k=K)
    t4 = pool.tile([PART, GF, 4], bf16, tag=tag + "4")
    nc.vector.tensor_add(t4[:], s3[:, :, 0:4], s3[:, :, 4:8])
    p2 = pool.tile([PART, GF, 2], bf16, tag=tag + "2")
    nc.vector.tensor_add(p2[:], t4[:, :, 0:2], t4[:, :, 2:4])
    q2 = pool.tile([PART, GF, 2], bf16, tag=tag + "q")
    nc.vector.tensor_add(q2[:], p2[:], s3[:, :, 8:10])
    nc.vector.tensor_add(out[:].unsqueeze(2), q2[:, :, 0:1], q2[:, :, 1:2])


def _build():
    if "nc" in _CACHE:
        return _CACHE["nc"]

    _patch_act_tables()
    f32 = mybir.dt.float32
    bf16 = mybir.dt.bfloat16
    nc = bacc.Bacc("TRN2", target_bir_lowering=False, debug=False, num_devices=NCORES)

    acts = nc.dram_tensor("acts", [NCHUNK, PART, FREE], f32, kind="ExternalInput").ap()
    labels = nc.dram_tensor(
        "labels", [NCHUNK, PART, PX_PER_PART], f32, kind="ExternalInput"
    ).ap()
    consts = nc.dram_tensor("consts", [C + 1], f32, kind="ExternalInput")
    stats_out = nc.dram_tensor(
        "stats", [PX_PER_PART * C, PX_PER_PART * EW], f32, kind="ExternalOutput"
    ).ap()
    stats2_out = nc.dram_tensor(
        "stats2", [PX_PER_PART * C, PX_PER_PART * G], f32, kind="ExternalOutput"
    ).ap()

    with tile.TileContext(nc) as tc:
        with ExitStack() as ctx:
            singles = ctx.enter_context(tc.tile_pool(name="singles", bufs=1))
            big = ctx.enter_context(tc.tile_pool(name="big", bufs=PH + 2))
            ebuf = ctx.enter_context(tc.tile_pool(name="ebuf", bufs=PH + 2))
            sybuf = ctx.enter_context(tc.tile_pool(name="sybuf", bufs=PH + 2))
            tree = ctx.enter_context(tc.tile_pool(name="tree", bufs=2))
            small = ctx.enter_context(tc.tile_pool(name="small", bufs=3))
            psum = ctx.enter_context(tc.tile_pool(name="psum", bufs=2, space="PSUM"))

            # constants: [1..8, 1.0] broadcast to all partitions
            cvec = singles.tile([PART, C + 1], f32)
            consts_b = bass.AP(tensor=consts, offset=0, ap=[[0, PART], [1, C + 1]])
            nc.sync.dma_start(out=cvec[:], in_=consts_b)
            iota_ps = psum.tile([PART, C], f32)
            nc.scalar.copy(out=iota_ps[:], in_=cvec[:, 0:C])
            mvec = singles.tile([PART, 1], f32)
            nc.vector.memset(mvec[:], -MSHIFT)

            stats_ps = psum.tile([PX_PER_PART * C, PX_PER_PART * EW], f32)
            stats2_ps = psum.tile([PX_PER_PART * C, PX_PER_PART * G], f32)

            for g0 in range(0, NCHUNK, PH):
                group = range(g0, min(g0 + PH, NCHUNK))
                a_t, e_t, sy_t = {}, {}, {}
                for ch in group:
                    a = big.tile([PART, FREE], f32, tag="a")
                    nc.sync.dma_start(out=a[:], in_=acts[ch])
                    a_t[ch] = a
                    e = ebuf.tile([PART, FREE], bf16, tag="e")
                    nc.scalar.activation(
                        out=e[:], in_=a[:], func=mybir.ActivationFunctionType.Exp
                    )
                    e_t[ch] = e
                for ch in group:
                    sy = sybuf.tile([PART, FREE], bf16, tag="sy")
                    nc.scalar.activation(
                        out=sy[:],
                        in_=a_t[ch][:],
                        func=mybir.ActivationFunctionType.Silu,
                        bias=mvec[:],
                    )
                    sy_t[ch] = sy
                for ch in group:
                    e, sy = e_t[ch], sy_t[ch]
                    lab = small.tile([PART, PX_PER_PART], f32, tag="lab")
                    nc.sync.dma_start(out=lab[:], in_=labels[ch])

                    z = small.tile([PART, GF], f32, tag="z")
                    _tree_groupsum(nc, tree, e, z, "z")

                    ss = small.tile([PART, GF], f32, tag="ss")
                    _tree_groupsum(nc, tree, sy, ss, "s")

                    # mask [128, j=8, c=8] = (label[j] == c+1)
                    mask = small.tile([PART, PX_PER_PART, C], f32, tag="mask")
                    lab_ap = lab[:]
                    lab_b = bass.AP(
                        tensor=lab_ap.tensor,
                        offset=lab_ap.offset,
                        ap=[lab_ap.ap[0], lab_ap.ap[1], [0, C]],
                    )
                    iota_ap = iota_ps[:]
                    iota_b = bass.AP(
                        tensor=iota_ap.tensor,
                        offset=iota_ap.offset,
                        ap=[iota_ap.ap[0], [0, PX_PER_PART], iota_ap.ap[1]],
                    )
                    nc.vector.tensor_tensor(
                        mask[:], lab_b, iota_b, mybir.AluOpType.is_equal
                    )

                    # lz tile [128, j=8, 33]: cols 0..31 = logZ, col 32 = 1.0
                    lz = small.tile([PART, PX_PER_PART, EW], f32, tag="lz")
                    nc.scalar.activation(
                        out=lz[:, :, 0:G],
                        in_=z[:].rearrange("p (j g) -> p j g", g=G),
                        func=mybir.ActivationFunctionType.Ln,
                    )
                    nc.vector.memset(lz[:, :, G : G + 1], 1.0)
                    # 1/Z on DVE (ACT is the bottleneck engine); approx is
                    # plenty: ent err ~ |meanx| * relerr << tolerance
                    rz = small.tile([PART, GF], f32, tag="rz")
                    nc.vector.reciprocal_approx_fast(out=rz[:], in_=z[:])

                    # meanx = (-SS) * rZ
                    meanx = small.tile([PART, GF], f32, tag="meanx")
                    nc.vector.scalar_tensor_tensor(
                        out=meanx[:],
                        in0=ss[:],
                        scalar=-1.0,
                        in1=rz[:],
                        op0=mybir.AluOpType.mult,
                        op1=mybir.AluOpType.mult,
                    )

                    nc.tensor.matmul(
                        out=stats_ps[:],
                        lhsT=mask[:].rearrange("p j c -> p (j c)"),
                        rhs=lz[:].rearrange("p j e -> p (j e)"),
                        start=(ch == 0),
                        stop=(ch == NCHUNK - 1),
                        skip_group_check=True,
                    )
                    nc.tensor.matmul(
                        out=stats2_ps[:],
                        lhsT=mask[:].rearrange("p j c -> p (j c)"),
                        rhs=meanx[:],
                        start=(ch == 0),
                        stop=(ch == NCHUNK - 1),
                        skip_group_check=True,
                    )

            stats_sb = singles.tile([PX_PER_PART * C, PX_PER_PART * EW], f32)
            nc.vector.tensor_copy(out=stats_sb[:], in_=stats_ps[:])
            nc.sync.dma_start(out=stats_out, in_=stats_sb[:])
            stats2_sb = singles.tile([PX_PER_PART * C, PX_PER_PART * G], f32)
            nc.vector.tensor_copy(out=stats2_sb[:], in_=stats2_ps[:])
            nc.sync.dma_start(out=stats2_out, in_=stats2_sb[:])

    nc.compile()
    _CACHE["nc"] = nc
    return nc


def _prep_inputs(prototype_activations, target_labels, proto_idx):
    acts = np.asarray(prototype_activations, dtype=np.float32)
    labels = np.asarray(target_labels)
    pidx = np.asarray(proto_idx)

    expected = np.arange(S * C * K, dtype=np.int64).reshape(S, C, K)
    if not np.array_equal(pidx.astype(np.int64), expected):
        # general (slow) fallback: permute proto columns on host
        acts = np.ascontiguousarray(acts[..., pidx.reshape(-1)])

    labels_f = labels.astype(np.float32)
    consts = np.concatenate(
        [np.arange(1, C + 1, dtype=np.float32), np.ones(1, dtype=np.float32)]
    )

    in_maps = []
    for b in range(B):
        in_maps.append(
            {
                "acts": np.ascontiguousarray(acts[b]).reshape(NCHUNK, PART, FREE),
                "labels": np.ascontiguousarray(labels_f[b]).reshape(
                    NCHUNK, PART, PX_PER_PART
                ),
                "consts": consts,
            }
        )
    return in_maps


def _combine(stats_list):
    """stats_list: per-core ([64, 264], [64, 256]) pairs -> final scalar."""
    num = np.zeros((B, S, C), dtype=np.float32)
    cnt = np.zeros((B, C), dtype=np.float32)
    jj = np.arange(PX_PER_PART)
    for b, (st1, st2) in enumerate(stats_list):
        st1 = st1.reshape(PX_PER_PART, C, PX_PER_PART, EW)  # [j, c, j', e]
        d1 = st1[jj, :, jj, :].sum(axis=0)  # [c, e]; e: s*C+c' | count
        st2 = st2.reshape(PX_PER_PART, C, PX_PER_PART, G)
        d2 = st2[jj, :, jj, :].sum(axis=0)  # [c, s*C+c'] of -SS/Z sums
        cntc = d1[:, S * C]
        # ent = logZ - e^m * SS/Z - m   (U = e^m*SS + m*Z)
        ent_cols = (
            d1[:, : S * C].reshape(C, S, C)
            + np.float32(np.exp(MSHIFT)) * d2.reshape(C, S, C)
            - np.float32(MSHIFT) * cntc[:, None, None].astype(np.float32)
        )
        num[b] = ent_cols[np.arange(C), :, np.arange(C)].T  # [s, c]
        cnt[b] = cntc
    num /= np.float32(np.log(K))
    present = cnt > 0
    mean_ent = num / np.maximum(cnt, 1.0)[:, None, :]
    n_entries = np.float32(present.sum() * S)
    total = np.float32((mean_ent * present[:, None, :]).sum(dtype=np.float64))
    if n_entries > 0:
        out = np.float32(total / max(n_entries, np.float32(1.0)))
    else:
        out = np.float32(0.0)
    return out


def kernel(prototype_activations, target_labels, proto_idx, _trace=False, _tmpdir=None):
    nc = _build()
    in_maps = _prep_inputs(prototype_activations, target_labels, proto_idx)
    res = run_bass_kernel_spmd(
        nc, in_maps, list(range(NCORES)), trace=_trace, tmpdir=_tmpdir
    )
    stats_list = [
        (res.results[i]["stats"], res.results[i]["stats2"]) for i in range(NCORES)
    ]
    out = _combine(stats_list)
    if _trace:
        return out, res
    return out



# revision 3
# speedup vs baseline: 1.9455x; 1.9455x over previous
"""Trainium2 Bass kernel for EntropySamplLoss, v7 (transposed PE-reduce).

Reference semantics (per image b):
  acts [N, P=320] viewed as [N, S=4, C=8, K=10] prototype groups
  ent[n, s, c] = normalized softmax entropy over the K protos of group (s, c)
  loss = mean over present (b, s, c) of (sum over pixels of class c of
         ent[n, s, c]) / count(c)

v7 layout (one image per NeuronCore, fp16):
  Host transposes acts to proto-major [640, M=N/2] fp16: row R = q*320 + P
  holds proto P of pixels with parity q (n = 2m + q).  Viewed as 5 DMA
  tiles [128, M].  With protos on partitions, the K=10 group sums become
  partition-axis reductions, done on the Tensor engine with fixed 0/1
  group-membership matrices gm[t] [128, 64] (g = q*32 + s*8 + c):

    per column-subchunk of 512 (PSUM bank), pairs stacked 64+64 rows:
      Z[g, m] = sum_t gm[t]^T @ exp(x_t)[:, m]       (PSUM accum, 5 matmuls)
      U[g, m] = sum_t gm[t]^T @ (x*exp(x))[:, m]     (5 matmuls)
      lnZ = Ln(Z)                                    (ACT)
      rZ  = reciprocal_approx_fast(Z)                (DVE)
      UrZ = U * rZ                                   (DVE scalar_tensor_tensor)
      num1 += sum_m mask*lnZ ; num2 += sum_m mask*UrZ (DVE tensor_tensor_reduce)
    host: ent-sums = (num1-num2)/ln(10), per-class means, final mean.

  exp runs once (no silu pass: U comes from a DVE mult), inputs are fp16
  (half the HBM traffic), and the old DVE tree-sums are gone (PE does them).
  Engine budgets/core: ACT ~170us (exp+ln), DVE ~165us (x*e^x + group ops),
  PE ~140us, DMA ~135us.  Baseline v6 (ACT-bound, 2 passes): 388us.
"""

import sys

if "/opt/trn_rl_repo" not in sys.path:
    sys.path.insert(0, "/opt/trn_rl_repo")

from contextlib import ExitStack

import numpy as np

import concourse.bacc as bacc
import concourse.bass as bass
import concourse.tile as tile
from concourse import mybir
from concourse.bass_utils import run_bass_kernel_spmd

# Problem shape (hardcoded per spec)
B, N, PP = 8, 65536, 320
S, C, K = 4, 8, 10
NCORES = 8

M = N // 2              # 32768 columns (column = even/odd pixel pair)
NT = 5                  # 640 transposed rows = 5 tiles of 128
SUB = 512               # PSUM-bank subchunk (512 f32 = one 2KB bank)
NSUB = M // SUB         # 64
NPAIR = NSUB // 2       # 32 stacked pairs
BCW = 4096              # big-chunk columns per DMA (1 MiB per tile DMA)
NBC = M // BCW          # 8
PAIRS_PER_BC = BCW // (2 * SUB)  # 4
G = 64                  # PSUM rows per subchunk: q(2) x s(4) x c(8)

_CACHE = {}


def _patch_act_tables():
    """Keep exp+ln in one ACT table set so no table switches are emitted."""
    import concourse.hw_specs as hw_specs

    tabs = hw_specs.get_activation_tables("gen3")
    E = mybir.ActivationFunctionType.Exp
    L = mybir.ActivationFunctionType.Ln
    for name, funcs in tabs.items():
        if name != "natural_log_exp_and_others":
            funcs.discard(E)
            funcs.discard(L)


def _group_matrices():
    """gm[t][p, g] = 1 iff transposed row R=128t+p belongs to PSUM row g."""
    gms = np.zeros((NT, 128, G), dtype=np.float16)
    for t in range(NT):
        for p in range(128):
            R = 128 * t + p
            q, P = divmod(R, PP)
            g = q * 32 + (P // 80) * 8 + (P % 80) // 10
            gms[t, p, g] = 1.0
    return gms


def _build():
    if "nc" in _CACHE:
        return _CACHE["nc"]

    _patch_act_tables()
    f32 = mybir.dt.float32
    f16 = mybir.dt.float16
    nc = bacc.Bacc("TRN2", target_bir_lowering=False, debug=False, num_devices=NCORES)

    acts_t = nc.dram_tensor("acts_t", [NT, 128, M], f16, kind="ExternalInput").ap()
    maskh = nc.dram_tensor("maskh", [128, M // 2], f16, kind="ExternalInput").ap()
    gmat = nc.dram_tensor("gmat", [NT, 128, G], f16, kind="ExternalInput").ap()
    parts1_out = nc.dram_tensor("parts1", [128, NPAIR], f32, kind="ExternalOutput").ap()
    parts2_out = nc.dram_tensor("parts2", [128, NPAIR], f32, kind="ExternalOutput").ap()

    with tile.TileContext(nc) as tc:
        with ExitStack() as ctx:
            singles = ctx.enter_context(tc.tile_pool(name="singles", bufs=1))
            xpool = ctx.enter_context(tc.tile_pool(name="xpool", bufs=2))
            epool = ctx.enter_context(tc.tile_pool(name="epool", bufs=2))
            mpool = ctx.enter_context(tc.tile_pool(name="mpool", bufs=2))
            spool = ctx.enter_context(tc.tile_pool(name="spool", bufs=3))
            psum = ctx.enter_context(tc.tile_pool(name="psum", bufs=2, space="PSUM"))

            gms = []
            for t in range(NT):
                gm = singles.tile([128, G], f16, name=f"gm{t}")
                nc.sync.dma_start(out=gm[:], in_=gmat[t])
                gms.append(gm)

            parts1 = singles.tile([128, NPAIR], f32)
            parts2 = singles.tile([128, NPAIR], f32)

            for bc in range(NBC):
                c0 = bc * BCW
                xs, es = [], []
                for t in range(NT):
                    x = xpool.tile([128, BCW], f16, tag=f"x{t}")
                    nc.sync.dma_start(out=x[:], in_=acts_t[t][:, c0 : c0 + BCW])
                    xs.append(x)
                mk = mpool.tile([128, BCW // 2], f16, tag="mk")
                nc.sync.dma_start(
                    out=mk[:], in_=maskh[:, c0 // 2 : c0 // 2 + BCW // 2]
                )
                for t in range(NT):
                    e = epool.tile([128, BCW], f16, tag=f"e{t}")
                    nc.scalar.activation(
                        out=e[:], in_=xs[t][:], func=mybir.ActivationFunctionType.Exp
                    )
                    es.append(e)
                for t in range(NT):
                    # x := x * e^x in place (the U-matmul moving tensor)
                    nc.vector.tensor_tensor(
                        xs[t][:], xs[t][:], es[t][:], mybir.AluOpType.mult
                    )

                for u in range(PAIRS_PER_BC):
                    pair = bc * PAIRS_PER_BC + u
                    zp = psum.tile([128, SUB], f32, tag="z")
                    up = psum.tile([128, SUB], f32, tag="u")
                    for blk in range(2):
                        lo = (2 * u + blk) * SUB
                        sl = slice(lo, lo + SUB)
                        zout = zp[64 * blk : 64 * blk + 64, :]
                        uout = up[64 * blk : 64 * blk + 64, :]
                        for t in range(NT):
                            nc.tensor.matmul(
                                out=zout,
                                lhsT=gms[t][:],
                                rhs=es[t][:, sl],
                                start=(t == 0),
                                stop=(t == NT - 1),
                                skip_group_check=True,
                            )
                            nc.tensor.matmul(
                                out=uout,
                                lhsT=gms[t][:],
                                rhs=xs[t][:, sl],
                                start=(t == 0),
                                stop=(t == NT - 1),
                                skip_group_check=True,
                            )

                    lnz = spool.tile([128, SUB], f16, tag="lnz")
                    nc.scalar.activation(
                        out=lnz[:], in_=zp[:], func=mybir.ActivationFunctionType.Ln
                    )
                    rz = spool.tile([128, SUB], f32, tag="rz")
                    nc.vector.reciprocal_approx_fast(out=rz[:], in_=zp[:])
                    urz = spool.tile([128, SUB], f16, tag="urz")
                    nc.vector.scalar_tensor_tensor(
                        out=urz[:],
                        in0=up[:],
                        scalar=1.0,
                        in1=rz[:],
                        op0=mybir.AluOpType.mult,
                        op1=mybir.AluOpType.mult,
                    )
                    msl = mk[:, u * SUB : (u + 1) * SUB]
                    d1 = spool.tile([128, SUB], f16, tag="d1")
                    nc.vector.scalar_tensor_tensor(
                        out=d1[:],
                        in0=lnz[:],
                        scalar=1.0,
                        in1=msl,
                        op0=mybir.AluOpType.mult,
                        op1=mybir.AluOpType.mult,
                        accum_out=parts1[:, pair : pair + 1],
                    )
                    d2 = spool.tile([128, SUB], f16, tag="d2")
                    nc.vector.scalar_tensor_tensor(
                        out=d2[:],
                        in0=urz[:],
                        scalar=1.0,
                        in1=msl,
                        op0=mybir.AluOpType.mult,
                        op1=mybir.AluOpType.mult,
                        accum_out=parts2[:, pair : pair + 1],
                    )

            nc.sync.dma_start(out=parts1_out, in_=parts1[:])
            nc.sync.dma_start(out=parts2_out, in_=parts2[:])

    nc.compile()
    _CACHE["nc"] = nc
    return nc


def _prep_inputs(prototype_activations, target_labels, proto_idx):
    acts = np.asarray(prototype_activations, dtype=np.float32)
    labels = np.asarray(target_labels)
    pidx = np.asarray(proto_idx)

    expected = np.arange(S * C * K, dtype=np.int64).reshape(S, C, K)
    if not np.array_equal(pidx.astype(np.int64), expected):
        # general (slow) fallback: permute proto columns on host
        acts = np.ascontiguousarray(acts[..., pidx.reshape(-1)])

    gms = _group_matrices()
    in_maps = []
    for b in range(B):
        x16 = acts[b].astype(np.float16)  # [N, 320]
        # [640, M]: row q*320+P = proto P of pixels n = 2m+q
        at = np.ascontiguousarray(
            x16.reshape(M, 2, PP).transpose(1, 2, 0)
        ).reshape(NT, 128, M)

        lab = labels[b].astype(np.int32)
        # L[q, u, blk, x] = label of pixel n = 2*(512*(2u+blk)+x) + q
        L = np.ascontiguousarray(lab.reshape(M, 2).T).reshape(2, NPAIR, 2, SUB)
        eq = L[:, :, :, :, None] == np.arange(1, C + 1, dtype=np.int32)
        # maskh[blk*64 + q*32 + s*8 + c, u*512 + x]
        mh = np.broadcast_to(
            eq.transpose(2, 0, 4, 1, 3)[:, :, None, :, :, :],
            (2, 2, S, C, NPAIR, SUB),
        ).astype(np.float16)
        in_maps.append(
            {
                "acts_t": at,
                "maskh": np.ascontiguousarray(mh).reshape(128, M // 2),
                "gmat": gms,
            }
        )
    return in_maps, labels


def _combine(parts_list, labels):
    """parts_list: per-core (parts1 [128, 32], parts2 [128, 32]) f32.
    Row = blk*64 + q*32 + s*8 + c, col = pair index."""
    num = np.zeros((B, S, C), dtype=np.float64)
    cnt = np.zeros((B, C), dtype=np.int64)
    for b, (p1, p2) in enumerate(parts_list):
        d = (p1.astype(np.float64) - p2.astype(np.float64)).sum(axis=1)
        num[b] = d.reshape(2, 2, S, C).sum(axis=(0, 1))
        lab = np.asarray(labels[b]).astype(np.int64)
        cnt[b] = np.bincount(lab, minlength=C + 1)[1 : C + 1]
    num /= np.log(K)
    present = cnt > 0
    mean_ent = num / np.maximum(cnt, 1)[:, None, :]
    n_entries = float(present.sum() * S)
    total = float((mean_ent * present[:, None, :]).sum())
    if n_entries > 0:
        return np.float32(total / max(n_entries, 1.0))
    return np.float32(0.0)


def kernel(prototype_activations, target_labels, proto_idx, _trace=False, _tmpdir=None):
    nc = _build()
    in_maps, labels = _prep_inputs(prototype_activations, target_labels, proto_idx)
    res = run_bass_kernel_spmd(
        nc, in_maps, list(range(NCORES)), trace=_trace, tmpdir=_tmpdir
    )
    parts_list = [
        (res.results[i]["parts1"], res.results[i]["parts2"]) for i in range(NCORES)
    ]
    out = _combine(parts_list, labels)
    if _trace:
        return out, res
    return out
